# revision 34
# baseline (speedup 1.0000x reference)
"""Block-circulant process via frequency-domain factorization on 8 cores.

out = x @ M factorizes through the (truncated, 48-bin) real FFT:
  stage A: per in-block j:  S[(fl p e), b] = sum_t F[t,(fl p e)] x[jB+t, b]
  stage M: per freq pair e: mid[(fl q i), b] = W_e[(fl p j),(fl q i)]^T G_e
  stage C: per out-block i: out[t, b] = sum_{fl q e} G[(fl q e), t] H_i

All device data is bf16 (host casts in/out; fp32 accumulation in PSUM).
The two partition-regroups bounce through DRAM: per chunk, 8 permuting
writes on GPSIMD/SWDGE (keeps ~6k descriptors of generation off the
HWDGE engines) + a few plain column-slice reads with 2KB descriptors.
Batch is split in CH=2 chunks so chunk k+1's stage A overlaps chunk
k's regroups and later stages. Matmul groups share one 2-bank PSUM
tile so one PSUM->SBUF copy serves 1KB rows; copies split across DVE
(*_lo tiles) and ACT (*_hi) -- Tile's dep tracking needs single-engine
writers per tile.

Sharding: pure data-parallel over batch (x dim 0), weights replicated.
"""

import numpy as np

B = 128
K_HALF = B // 2 + 1  # 65
KT = 48  # frequency truncation
KI = 32
KO = 32
BATCH = 4096
IN_F = 4096
OUT_F = 4096

N_CORES = 8
BQ = BATCH // N_CORES  # 512 batch columns per core
NP = KT // 2  # 24 frequency pairs

CH = 2           # batch chunks per core (pipeline depth)
BC = BQ // CH    # 256 batch columns per chunk
MMG = 1024 // BC  # matmuls per 2-bank PSUM tile (4)
EQ = 12          # e's per regroup-1 read DMA (2 reads/chunk)
IQ = 16          # i's per regroup-2 read DMA (2 reads/chunk)
JG = 8           # j's per x-load DMA / i's per out-store DMA

_CACHE = {}
LAST_RESULTS = None
TRACE = False


def _build_nc():
    import concourse.bacc as bacc
    import concourse.mybir as mybir
    import concourse.tile as tile

    BF16 = mybir.dt.bfloat16

    nc = bacc.Bacc(None, target_bir_lowering=False)
    # x packed on host: [t, (c, j, b)]
    xP = nc.declare_dram_parameter("xP", [128, CH * KI * BC], BF16,
                                   isOutput=False)
    fmat = nc.declare_dram_parameter("fmat", [128, 96], BF16, isOutput=False)
    gmat = nc.declare_dram_parameter("gmat", [96, 128], BF16, isOutput=False)
    wmid = nc.declare_dram_parameter("wmid", [128, NP * 128], BF16,
                                     isOutput=False)
    # out packed: [t, (c, i, b)]; host unpacks + upcasts
    oP = nc.declare_dram_parameter("oP", [128, CH * KO * BC], BF16,
                                   isOutput=True)

    # DRAM scratch for the two partition-regroups.
    # sS_g[(flp j), (c e b)] ; sM_h[(flq e), (c i b)]
    sS_g = nc.dram_tensor("sS_g", [128, CH * NP * BC], BF16)
    sM_h = nc.dram_tensor("sM_h", [96, CH * KO * BC], BF16)
    sS_w = sS_g.rearrange("(flp j) (c e b) -> c flp e j b", flp=4, j=KI,
                          c=CH, e=NP)
    sM_w = sM_h.rearrange("(flq e) (c i b) -> c flq i e b", flq=4, e=NP,
                          c=CH, i=KO)

    JH = KI // 2  # 16 j's per s half
    EH = NP // 2  # 12 e's per mid half

    with tile.TileContext(nc) as tc:
        with (
            tc.tile_pool(name="cpool", bufs=1) as cpool,
            tc.tile_pool(name="xpool", bufs=CH * (KI // JG) + 1) as xpool,
            tc.tile_pool(name="spool", bufs=2) as spool,
            tc.tile_pool(name="gpool", bufs=NP // EQ + 2) as gpool,
            tc.tile_pool(name="mpool", bufs=2) as mpool,
            tc.tile_pool(name="hpool", bufs=KO // IQ + 2) as hpool,
            tc.tile_pool(name="opool", bufs=4) as opool,
            tc.tile_pool(name="psum", bufs=4, space="PSUM") as psum,
        ):
            f_t = cpool.tile([128, 96], BF16, name="f_t")
            nc.scalar.dma_start(f_t[:], fmat[:])
            g_t = cpool.tile([96, 128], BF16, name="g_t")
            nc.scalar.dma_start(g_t[:], gmat[:])
            w_all = cpool.tile([128, NP * 128], BF16, name="w_all")
            nc.scalar.dma_start(w_all[:], wmid[:])

            # ---- software-pipelined emission: PE stream is
            # A0 A1 M0 M1 C0 C1, with regroup DMA data flowing behind the
            # PE. Regroup-2 writes + reads go on the otherwise idle GPSIMD
            # (SWDGE) queue; sync carries x/sW/gR/out; ACT and DVE split
            # the PSUM->SBUF copies via single-writer tiles.
            x_g = {}
            for c in range(CH):
                for g in range(KI // JG):
                    xt = xpool.tile([128, JG * BC], BF16, name="x_g",
                                    tag="x_g")
                    col0 = (c * KI + g * JG) * BC
                    nc.sync.dma_start(xt[:], xP[:, col0:col0 + JG * BC])
                    x_g[c, g] = xt

            s_t = {}

            def stage_a(c):
                s_lo = spool.tile([96, JH * BC], BF16, name="s_lo",
                                  tag="s_lo")
                s_hi = spool.tile([96, JH * BC], BF16, name="s_hi",
                                  tag="s_hi")
                for p4 in range(KI // MMG):
                    ps = psum.tile([96, MMG * BC], mybir.dt.float32,
                                   name="ps_a", tag="ps")
                    for k in range(MMG // 2):
                        j = MMG * p4 + 2 * k
                        rhs = x_g[c, j // JG][
                            :, (j % JG) * BC:(j % JG + 2) * BC]
                        nc.tensor.matmul(ps[:, 2 * k * BC:(2 * k + 2) * BC],
                                         f_t[:], rhs, start=True, stop=True)
                    half = KI // MMG // 2
                    if p4 < half:
                        nc.vector.tensor_copy(
                            s_lo[:, p4 * MMG * BC:(p4 + 1) * MMG * BC],
                            ps[:])
                    else:
                        c0 = (p4 - half) * MMG * BC
                        nc.scalar.copy(s_hi[:, c0:c0 + MMG * BC], ps[:])
                s_t[c] = (s_lo, s_hi)

            def emit_sw(c):
                eng = nc.sync
                for hh, sh in enumerate(s_t[c]):
                    for flp in range(4):
                        eng.dma_start(
                            sS_w[c, flp, :, hh * JH:(hh + 1) * JH],
                            sh[flp * 24:(flp + 1) * 24, :].rearrange(
                                "e (j b) -> e j b", j=JH))

            g_q = {}

            def emit_gr(c):
                eng = nc.sync
                for qd in range(NP // EQ):
                    gq = gpool.tile([128, EQ * BC], BF16, name="g_q",
                                    tag="g_q")
                    col0 = (c * NP + qd * EQ) * BC
                    eng.dma_start(gq[:], sS_g[:, col0:col0 + EQ * BC])
                    g_q[c, qd] = gq

            m_t = {}

            def stage_m(c):
                m_lo = mpool.tile([128, EH * BC], BF16, name="m_lo",
                                  tag="m_lo")
                m_hi = mpool.tile([128, EH * BC], BF16, name="m_hi",
                                  tag="m_hi")
                for p4 in range(NP // MMG):
                    ps = psum.tile([128, MMG * BC], mybir.dt.float32,
                                   name="ps_m", tag="ps")
                    for k in range(MMG):
                        e = MMG * p4 + k
                        gq = g_q[c, e // EQ]
                        # W_e is fl-block-diagonal: run the two 64-wide
                        # halves concurrently in disjoint PE quadrants
                        for fl in range(2):
                            r0 = fl * 64
                            rhs = gq[r0:r0 + 64,
                                     (e % EQ) * BC:(e % EQ + 1) * BC]
                            lhsT = w_all[r0:r0 + 64,
                                         e * 128 + r0:e * 128 + r0 + 64]
                            nc.tensor.matmul(
                                ps[r0:r0 + 64, k * BC:(k + 1) * BC],
                                lhsT, rhs, start=True, stop=True,
                                tile_position=(r0, r0))
                    half = NP // MMG // 2
                    if p4 < half:
                        nc.vector.tensor_copy(
                            m_lo[:, p4 * MMG * BC:(p4 + 1) * MMG * BC],
                            ps[:])
                    else:
                        c0 = (p4 - half) * MMG * BC
                        nc.scalar.copy(m_hi[:, c0:c0 + MMG * BC], ps[:])
                m_t[c] = (m_lo, m_hi)

            def emit_mw(c):
                for hh, mh in enumerate(m_t[c]):
                    for flq in range(4):
                        nc.gpsimd.dma_start(
                            sM_w[c, flq, :, hh * EH:(hh + 1) * EH],
                            mh[flq * 32:(flq + 1) * 32, :].rearrange(
                                "i (e b) -> i e b", e=EH))

            h_q = {}

            def emit_hr(c):
                for qd in range(KO // IQ):
                    hq = hpool.tile([96, IQ * BC], BF16, name="h_q",
                                    tag="h_q")
                    col0 = (c * KO + qd * IQ) * BC
                    nc.gpsimd.dma_start(hq[:],
                                        sM_h[:, col0:col0 + IQ * BC])
                    h_q[c, qd] = hq

            def stage_c(c):
                for p4 in range(KO // MMG):
                    o_g = opool.tile([128, MMG * BC], BF16, name="o_g",
                                     tag="o_g")
                    ps = psum.tile([128, MMG * BC], mybir.dt.float32,
                                   name="ps_c", tag="ps")
                    for k in range(MMG // 2):
                        i = MMG * p4 + 2 * k
                        rhs = h_q[c, i // IQ][
                            :, (i % IQ) * BC:(i % IQ + 2) * BC]
                        nc.tensor.matmul(ps[:, 2 * k * BC:(2 * k + 2) * BC],
                                         g_t[:], rhs, start=True, stop=True)
                    if p4 % 2 == 0:
                        nc.vector.tensor_copy(o_g[:], ps[:])
                    else:
                        nc.scalar.copy(o_g[:], ps[:])
                    col0 = (c * KO + p4 * MMG) * BC
                    nc.scalar.dma_start(oP[:, col0:col0 + MMG * BC],
                                        o_g[:])

            stage_a(0)
            emit_sw(0)
            stage_a(1)
            emit_gr(0)
            stage_m(0)
            emit_sw(1)
            emit_mw(0)
            emit_gr(1)
            stage_m(1)
            emit_hr(0)
            stage_c(0)
            emit_mw(1)
            emit_hr(1)
            stage_c(1)
    nc.finalize()
    return nc


def _get_nc():
    if "nc" not in _CACHE:
        _CACHE["nc"] = _build_nc()
    return _CACHE["nc"]


def _host_weights(W_real, W_imag):
    """F [128,96], G [96,128], Wmid [24,128,128] (float64)."""
    t = np.arange(B)[:, None].astype(np.float64)
    # F columns ordered (fl, p, e): f = 2e + fl; p=0 -> cos, p=1 -> -sin.
    F = np.zeros((128, 96))
    for fl in range(2):
        for p in range(2):
            for e in range(NP):
                f = 2 * e + fl
                col = fl * 48 + p * 24 + e
                w = 2 * np.pi * f * t[:, 0] / B
                F[:, col] = np.cos(w) if p == 0 else -np.sin(w)
    # G rows ordered (fl, q, e) with f = 2e + fl: q=0 -> scale*cos,
    # q=1 -> -scale*sin
    G = np.zeros((96, 128))
    scale = np.full(KT, 2.0 / B)
    scale[0] = 1.0 / B
    for fl in range(2):
        for q in range(2):
            for e in range(NP):
                f = 2 * e + fl
                w = 2 * np.pi * f * np.arange(B) / B
                G[fl * 48 + q * 24 + e] = (scale[f] * np.cos(w) if q == 0
                                           else -scale[f] * np.sin(w))
    # Wmid[e]: rows (fl, p, j), cols (fl, q, i); block-diag in fl
    Wr = W_real.astype(np.float64)
    Wi = W_imag.astype(np.float64)
    Wm = np.zeros((NP, 128, 128))
    for e in range(NP):
        for fl in range(2):
            f = 2 * e + fl
            r0, c0 = fl * 64, fl * 64
            # q=0: Re_out = Wr @ Re + Wi @ Im ; q=1: Im_out = Wr @ Im - Wi @ Re
            Wrf = Wr[:, :, f].T  # [j, i]
            Wif = Wi[:, :, f].T
            Wm[e, r0:r0 + 32, c0:c0 + 32] = Wrf            # p0 -> q0: Wr
            Wm[e, r0 + 32:r0 + 64, c0:c0 + 32] = Wif       # p1 -> q0: Wi
            Wm[e, r0:r0 + 32, c0 + 32:c0 + 64] = -Wif      # p0 -> q1: -Wi
            Wm[e, r0 + 32:r0 + 64, c0 + 32:c0 + 64] = Wrf  # p1 -> q1: Wr
    return F, G, Wm


def kernel(x, W_real, W_imag):
    global LAST_RESULTS
    import ml_dtypes
    from concourse.bass_utils import run_bass_kernel_spmd

    bf16 = np.dtype(ml_dtypes.bfloat16)
    x = np.asarray(x, dtype=np.float32)
    F, G, Wm = _host_weights(np.asarray(W_real), np.asarray(W_imag))
    Fb = np.ascontiguousarray(F).astype(bf16)
    Gb = np.ascontiguousarray(G).astype(bf16)
    wm_packed = np.ascontiguousarray(
        Wm.transpose(1, 0, 2).reshape(128, NP * 128)).astype(bf16)

    # pack x: [batch, (j t)] -> per core [t, (c, j, b)]
    xr = x.reshape(N_CORES, CH, BC, KI, B).transpose(0, 4, 1, 3, 2)
    xr = np.ascontiguousarray(xr.reshape(N_CORES, B, CH * KI * BC)).astype(
        bf16)

    in_maps = []
    for core in range(N_CORES):
        in_maps.append({"xP": xr[core], "fmat": Fb, "gmat": Gb,
                        "wmid": wm_packed})

    nc = _get_nc()
    res = run_bass_kernel_spmd(nc, in_maps, list(range(N_CORES)), trace=TRACE)
    LAST_RESULTS = res

    out = np.empty((BATCH, OUT_F), np.float32)
    for core in range(N_CORES):
        oPc = np.asarray(res.results[core]["oP"]).astype(np.float32)
        # [t, (c, i, b)] -> [(c b), (i t)]
        oc = oPc.reshape(B, CH, KO, BC).transpose(1, 3, 2, 0)
        out[core * BQ:(core + 1) * BQ, :] = oc.reshape(BQ, OUT_F)
    return out


# revision 35
# speedup vs baseline: 1.0311x; 1.0311x over previous
"""Block-circulant process via frequency-domain factorization on 8 cores.

out = x @ M factorizes through the (truncated, 48-bin) real FFT:
  stage A: per in-block j:  S[(fl p e), b] = sum_t F[t,(fl p e)] x[jB+t, b]
  stage M: per freq pair e: mid[(fl q i), b] = W_e[(fl p j),(fl q i)]^T G_e
  stage C: per out-block i: out[t, b] = sum_{fl q e} G[(fl q e), t] H_i

All device data is bf16 (host casts in/out; fp32 accumulation in PSUM).
The two partition-regroups bounce through DRAM: per chunk, 8 permuting
writes on GPSIMD/SWDGE (keeps ~6k descriptors of generation off the
HWDGE engines) + a few plain column-slice reads with 2KB descriptors.
Batch is split in CH=2 chunks so chunk k+1's stage A overlaps chunk
k's regroups and later stages. Matmul groups share one 2-bank PSUM
tile so one PSUM->SBUF copy serves 1KB rows; copies split across DVE
(*_lo tiles) and ACT (*_hi) -- Tile's dep tracking needs single-engine
writers per tile.

Sharding: pure data-parallel over batch (x dim 0), weights replicated.
"""

import numpy as np

B = 128
K_HALF = B // 2 + 1  # 65
KT = 48  # frequency truncation
KI = 32
KO = 32
BATCH = 4096
IN_F = 4096
OUT_F = 4096

N_CORES = 8
BQ = BATCH // N_CORES  # 512 batch columns per core
NP = KT // 2  # 24 frequency pairs

CH = 2           # batch chunks per core (pipeline depth)
BC = BQ // CH    # 256 batch columns per chunk
MMG = 1024 // BC  # matmuls per 2-bank PSUM tile (4)
EQ = 12          # e's per regroup-1 read DMA (2 reads/chunk)
IQ = 16          # i's per regroup-2 read DMA (2 reads/chunk)
JG = 8           # j's per x-load DMA / i's per out-store DMA

_CACHE = {}
LAST_RESULTS = None
TRACE = False


def _build_nc():
    import concourse.bacc as bacc
    import concourse.mybir as mybir
    import concourse.tile as tile

    BF16 = mybir.dt.bfloat16

    nc = bacc.Bacc(None, target_bir_lowering=False)
    # x packed on host: [t, (c, j, b)]
    xP = nc.declare_dram_parameter("xP", [128, CH * KI * BC], BF16,
                                   isOutput=False)
    fmat = nc.declare_dram_parameter("fmat", [128, 96], BF16, isOutput=False)
    gmat = nc.declare_dram_parameter("gmat", [96, 128], BF16, isOutput=False)
    wmid = nc.declare_dram_parameter("wmid", [128, NP * 128], BF16,
                                     isOutput=False)
    # out packed: [t, (c, i, b)]; host unpacks + upcasts
    oP = nc.declare_dram_parameter("oP", [128, CH * KO * BC], BF16,
                                   isOutput=True)

    # DRAM scratch for the two partition-regroups.
    # sS_g[(flp j), (c e b)] ; sM_h[(flq e), (c i b)]
    sS_g = nc.dram_tensor("sS_g", [128, CH * NP * BC], BF16)
    sM_h = nc.dram_tensor("sM_h", [96, CH * KO * BC], BF16)
    sS_w = sS_g.rearrange("(flp j) (c e b) -> c flp e j b", flp=4, j=KI,
                          c=CH, e=NP)
    sM_w = sM_h.rearrange("(flq e) (c i b) -> c flq i e b", flq=4, e=NP,
                          c=CH, i=KO)

    JH = KI // 2  # 16 j's per s half
    EH = NP // 2  # 12 e's per mid half

    with tile.TileContext(nc) as tc:
        with (
            tc.tile_pool(name="cpool", bufs=1) as cpool,
            tc.tile_pool(name="xpool", bufs=CH * (KI // JG) + 1) as xpool,
            tc.tile_pool(name="spool", bufs=2) as spool,
            tc.tile_pool(name="gpool", bufs=NP // EQ + 2) as gpool,
            tc.tile_pool(name="mpool", bufs=2) as mpool,
            tc.tile_pool(name="hpool", bufs=KO // IQ + 2) as hpool,
            tc.tile_pool(name="opool", bufs=3) as opool,
            tc.tile_pool(name="psum", bufs=4, space="PSUM") as psum,
        ):
            f_t = cpool.tile([128, 96], BF16, name="f_t")
            nc.scalar.dma_start(f_t[:], fmat[:])
            g_t = cpool.tile([96, 128], BF16, name="g_t")
            nc.scalar.dma_start(g_t[:], gmat[:])
            w_all = cpool.tile([128, NP * 128], BF16, name="w_all")
            nc.scalar.dma_start(w_all[:], wmid[:])

            # ---- software-pipelined emission: PE stream is
            # A0 A1 M0 M1 C0 C1, with regroup DMA data flowing behind the
            # PE. Regroup-2 writes + reads go on the otherwise idle GPSIMD
            # (SWDGE) queue; sync carries x/sW/gR/out; ACT and DVE split
            # the PSUM->SBUF copies via single-writer tiles.
            x_g = {}
            for c in range(CH):
                for g in range(KI // JG):
                    xt = xpool.tile([128, JG * BC], BF16, name="x_g",
                                    tag="x_g")
                    col0 = (c * KI + g * JG) * BC
                    nc.sync.dma_start(xt[:], xP[:, col0:col0 + JG * BC])
                    x_g[c, g] = xt

            s_t = {}

            def stage_a(c):
                s_lo = spool.tile([96, JH * BC], BF16, name="s_lo",
                                  tag="s_lo")
                s_hi = spool.tile([96, JH * BC], BF16, name="s_hi",
                                  tag="s_hi")
                for p4 in range(KI // MMG):
                    ps = psum.tile([96, MMG * BC], mybir.dt.float32,
                                   name="ps_a", tag="ps")
                    for k in range(MMG // 2):
                        j = MMG * p4 + 2 * k
                        rhs = x_g[c, j // JG][
                            :, (j % JG) * BC:(j % JG + 2) * BC]
                        nc.tensor.matmul(ps[:, 2 * k * BC:(2 * k + 2) * BC],
                                         f_t[:], rhs, start=True, stop=True)
                    half = KI // MMG // 2
                    if p4 < half:
                        nc.vector.tensor_copy(
                            s_lo[:, p4 * MMG * BC:(p4 + 1) * MMG * BC],
                            ps[:])
                    else:
                        c0 = (p4 - half) * MMG * BC
                        nc.scalar.copy(s_hi[:, c0:c0 + MMG * BC], ps[:])
                s_t[c] = (s_lo, s_hi)

            def emit_sw(c):
                eng = nc.sync
                for hh, sh in enumerate(s_t[c]):
                    for flp in range(4):
                        eng.dma_start(
                            sS_w[c, flp, :, hh * JH:(hh + 1) * JH],
                            sh[flp * 24:(flp + 1) * 24, :].rearrange(
                                "e (j b) -> e j b", j=JH))

            g_q = {}

            def emit_gr(c):
                eng = nc.sync
                for qd in range(NP // EQ):
                    gq = gpool.tile([128, EQ * BC], BF16, name="g_q",
                                    tag="g_q")
                    col0 = (c * NP + qd * EQ) * BC
                    eng.dma_start(gq[:], sS_g[:, col0:col0 + EQ * BC])
                    g_q[c, qd] = gq

            m_t = {}

            def stage_m(c):
                m_lo = mpool.tile([128, EH * BC], BF16, name="m_lo",
                                  tag="m_lo")
                m_hi = mpool.tile([128, EH * BC], BF16, name="m_hi",
                                  tag="m_hi")
                for p4 in range(NP // MMG):
                    ps = psum.tile([128, MMG * BC], mybir.dt.float32,
                                   name="ps_m", tag="ps")
                    for k in range(MMG):
                        e = MMG * p4 + k
                        gq = g_q[c, e // EQ]
                        # W_e is fl-block-diagonal: run the two 64-wide
                        # halves concurrently in disjoint PE quadrants
                        for fl in range(2):
                            r0 = fl * 64
                            rhs = gq[r0:r0 + 64,
                                     (e % EQ) * BC:(e % EQ + 1) * BC]
                            lhsT = w_all[r0:r0 + 64,
                                         e * 128 + r0:e * 128 + r0 + 64]
                            nc.tensor.matmul(
                                ps[r0:r0 + 64, k * BC:(k + 1) * BC],
                                lhsT, rhs, start=True, stop=True,
                                tile_position=(r0, r0))
                    half = NP // MMG // 2
                    if p4 < half:
                        nc.vector.tensor_copy(
                            m_lo[:, p4 * MMG * BC:(p4 + 1) * MMG * BC],
                            ps[:])
                    else:
                        c0 = (p4 - half) * MMG * BC
                        nc.scalar.copy(m_hi[:, c0:c0 + MMG * BC], ps[:])
                m_t[c] = (m_lo, m_hi)

            def emit_mw(c):
                for hh, mh in enumerate(m_t[c]):
                    for flq in range(4):
                        nc.gpsimd.dma_start(
                            sM_w[c, flq, :, hh * EH:(hh + 1) * EH],
                            mh[flq * 32:(flq + 1) * 32, :].rearrange(
                                "i (e b) -> i e b", e=EH))

            h_q = {}

            def emit_hr(c):
                for qd in range(KO // IQ):
                    hq = hpool.tile([96, IQ * BC], BF16, name="h_q",
                                    tag="h_q")
                    col0 = (c * KO + qd * IQ) * BC
                    nc.gpsimd.dma_start(hq[:],
                                        sM_h[:, col0:col0 + IQ * BC])
                    h_q[c, qd] = hq

            def stage_c(c):
                o_g = None
                for p4 in range(KO // MMG):
                    if (p4 * MMG) % JG == 0:
                        o_g = opool.tile([128, JG * BC], BF16, name="o_g",
                                         tag="o_g")
                    ps = psum.tile([128, MMG * BC], mybir.dt.float32,
                                   name="ps_c", tag="ps")
                    for k in range(MMG // 2):
                        i = MMG * p4 + 2 * k
                        rhs = h_q[c, i // IQ][
                            :, (i % IQ) * BC:(i % IQ + 2) * BC]
                        nc.tensor.matmul(ps[:, 2 * k * BC:(2 * k + 2) * BC],
                                         g_t[:], rhs, start=True, stop=True)
                    c0 = ((p4 * MMG) % JG) * BC
                    if ((p4 * MMG) // JG) % 2 == 0:
                        nc.vector.tensor_copy(o_g[:, c0:c0 + MMG * BC],
                                              ps[:])
                    else:
                        nc.scalar.copy(o_g[:, c0:c0 + MMG * BC], ps[:])
                    if ((p4 + 1) * MMG) % JG == 0:
                        col0 = (c * KO + (p4 + 1) * MMG - JG) * BC
                        nc.sync.dma_start(oP[:, col0:col0 + JG * BC],
                                          o_g[:])

            stage_a(0)
            emit_sw(0)
            stage_a(1)
            emit_gr(0)
            stage_m(0)
            emit_sw(1)
            emit_mw(0)
            emit_gr(1)
            stage_m(1)
            emit_hr(0)
            stage_c(0)
            emit_mw(1)
            emit_hr(1)
            stage_c(1)
    nc.finalize()
    return nc


def _get_nc():
    if "nc" not in _CACHE:
        _CACHE["nc"] = _build_nc()
    return _CACHE["nc"]


def _host_weights(W_real, W_imag):
    """F [128,96], G [96,128], Wmid [24,128,128] (float64)."""
    t = np.arange(B)[:, None].astype(np.float64)
    # F columns ordered (fl, p, e): f = 2e + fl; p=0 -> cos, p=1 -> -sin.
    F = np.zeros((128, 96))
    for fl in range(2):
        for p in range(2):
            for e in range(NP):
                f = 2 * e + fl
                col = fl * 48 + p * 24 + e
                w = 2 * np.pi * f * t[:, 0] / B
                F[:, col] = np.cos(w) if p == 0 else -np.sin(w)
    # G rows ordered (fl, q, e) with f = 2e + fl: q=0 -> scale*cos,
    # q=1 -> -scale*sin
    G = np.zeros((96, 128))
    scale = np.full(KT, 2.0 / B)
    scale[0] = 1.0 / B
    for fl in range(2):
        for q in range(2):
            for e in range(NP):
                f = 2 * e + fl
                w = 2 * np.pi * f * np.arange(B) / B
                G[fl * 48 + q * 24 + e] = (scale[f] * np.cos(w) if q == 0
                                           else -scale[f] * np.sin(w))
    # Wmid[e]: rows (fl, p, j), cols (fl, q, i); block-diag in fl
    Wr = W_real.astype(np.float64)
    Wi = W_imag.astype(np.float64)
    Wm = np.zeros((NP, 128, 128))
    for e in range(NP):
        for fl in range(2):
            f = 2 * e + fl
            r0, c0 = fl * 64, fl * 64
            # q=0: Re_out = Wr @ Re + Wi @ Im ; q=1: Im_out = Wr @ Im - Wi @ Re
            Wrf = Wr[:, :, f].T  # [j, i]
            Wif = Wi[:, :, f].T
            Wm[e, r0:r0 + 32, c0:c0 + 32] = Wrf            # p0 -> q0: Wr
            Wm[e, r0 + 32:r0 + 64, c0:c0 + 32] = Wif       # p1 -> q0: Wi
            Wm[e, r0:r0 + 32, c0 + 32:c0 + 64] = -Wif      # p0 -> q1: -Wi
            Wm[e, r0 + 32:r0 + 64, c0 + 32:c0 + 64] = Wrf  # p1 -> q1: Wr
    return F, G, Wm


def kernel(x, W_real, W_imag):
    global LAST_RESULTS
    import ml_dtypes
    from concourse.bass_utils import run_bass_kernel_spmd

    bf16 = np.dtype(ml_dtypes.bfloat16)
    x = np.asarray(x, dtype=np.float32)
    F, G, Wm = _host_weights(np.asarray(W_real), np.asarray(W_imag))
    Fb = np.ascontiguousarray(F).astype(bf16)
    Gb = np.ascontiguousarray(G).astype(bf16)
    wm_packed = np.ascontiguousarray(
        Wm.transpose(1, 0, 2).reshape(128, NP * 128)).astype(bf16)

    # pack x: [batch, (j t)] -> per core [t, (c, j, b)]
    xr = x.reshape(N_CORES, CH, BC, KI, B).transpose(0, 4, 1, 3, 2)
    xr = np.ascontiguousarray(xr.reshape(N_CORES, B, CH * KI * BC)).astype(
        bf16)

    in_maps = []
    for core in range(N_CORES):
        in_maps.append({"xP": xr[core], "fmat": Fb, "gmat": Gb,
                        "wmid": wm_packed})

    nc = _get_nc()
    res = run_bass_kernel_spmd(nc, in_maps, list(range(N_CORES)), trace=TRACE)
    LAST_RESULTS = res

    out = np.empty((BATCH, OUT_F), np.float32)
    for core in range(N_CORES):
        oPc = np.asarray(res.results[core]["oP"]).astype(np.float32)
        # [t, (c, i, b)] -> [(c b), (i t)]
        oc = oPc.reshape(B, CH, KO, BC).transpose(1, 3, 2, 0)
        out[core * BQ:(core + 1) * BQ, :] = oc.reshape(BQ, OUT_F)
    return out


# revision 36
# speedup vs baseline: 1.0563x; 1.0244x over previous
"""Block-circulant process via frequency-domain factorization on 8 cores.

out = x @ M factorizes through the (truncated, 48-bin) real FFT:
  stage A: per in-block j:  S[(fl p e), b] = sum_t F[t,(fl p e)] x[jB+t, b]
  stage M: per freq pair e: mid[(fl q i), b] = W_e[(fl p j),(fl q i)]^T G_e
  stage C: per out-block i: out[t, b] = sum_{fl q e} G[(fl q e), t] H_i

All device data is bf16 (host casts in/out; fp32 accumulation in PSUM).
The two partition-regroups bounce through DRAM: per chunk, 8 permuting
writes on GPSIMD/SWDGE (keeps ~6k descriptors of generation off the
HWDGE engines) + a few plain column-slice reads with 2KB descriptors.
Batch is split in CH=2 chunks so chunk k+1's stage A overlaps chunk
k's regroups and later stages. Matmul groups share one 2-bank PSUM
tile so one PSUM->SBUF copy serves 1KB rows; copies split across DVE
(*_lo tiles) and ACT (*_hi) -- Tile's dep tracking needs single-engine
writers per tile.

Sharding: pure data-parallel over batch (x dim 0), weights replicated.
"""

import numpy as np

B = 128
K_HALF = B // 2 + 1  # 65
KT = 48  # frequency truncation
KI = 32
KO = 32
BATCH = 4096
IN_F = 4096
OUT_F = 4096

N_CORES = 8
BQ = BATCH // N_CORES  # 512 batch columns per core
NP = KT // 2  # 24 frequency pairs

CH = 2           # batch chunks per core (pipeline depth)
BC = BQ // CH    # 256 batch columns per chunk
MMG = 1024 // BC  # matmuls per 2-bank PSUM tile (4)
EQ = 12          # e's per regroup-1 read DMA (2 reads/chunk)
IQ = 16          # i's per regroup-2 read DMA (2 reads/chunk)
JG = 8           # j's per x-load DMA / i's per out-store DMA

_CACHE = {}
LAST_RESULTS = None
TRACE = False


def _build_nc():
    import concourse.bacc as bacc
    import concourse.mybir as mybir
    import concourse.tile as tile

    BF16 = mybir.dt.bfloat16

    nc = bacc.Bacc(None, target_bir_lowering=False)
    # x packed on host: [t, (c, j, b)]
    xP = nc.declare_dram_parameter("xP", [128, CH * KI * BC], BF16,
                                   isOutput=False)
    fmat = nc.declare_dram_parameter("fmat", [128, 96], BF16, isOutput=False)
    gmat = nc.declare_dram_parameter("gmat", [96, 128], BF16, isOutput=False)
    wmid = nc.declare_dram_parameter("wmid", [128, NP * 128], BF16,
                                     isOutput=False)
    # out packed: [t, (c, i, b)]; host unpacks + upcasts
    oP = nc.declare_dram_parameter("oP", [128, CH * KO * BC], BF16,
                                   isOutput=True)

    # DRAM scratch for the two partition-regroups.
    # sS_g[(flp j), (c e b)] ; sM_h[(flq e), (c i b)]
    sS_g = nc.dram_tensor("sS_g", [128, CH * NP * BC], BF16)
    sM_h = nc.dram_tensor("sM_h", [96, CH * KO * BC], BF16)
    sS_w = sS_g.rearrange("(flp j) (c e b) -> c flp e j b", flp=4, j=KI,
                          c=CH, e=NP)
    sM_w = sM_h.rearrange("(flq e) (c i b) -> c flq i e b", flq=4, e=NP,
                          c=CH, i=KO)

    JH = KI // 2  # 16 j's per s half
    EH = NP // 2  # 12 e's per mid half

    with tile.TileContext(nc) as tc:
        with (
            tc.tile_pool(name="cpool", bufs=1) as cpool,
            tc.tile_pool(name="xpool", bufs=CH * (KI // JG) + 1) as xpool,
            tc.tile_pool(name="spool", bufs=2) as spool,
            tc.tile_pool(name="gpool", bufs=NP // EQ + 2) as gpool,
            tc.tile_pool(name="mpool", bufs=2) as mpool,
            tc.tile_pool(name="hpool", bufs=KO // IQ + 2) as hpool,
            tc.tile_pool(name="opool", bufs=3) as opool,
            tc.tile_pool(name="psum", bufs=4, space="PSUM") as psum,
        ):
            f_t = cpool.tile([128, 96], BF16, name="f_t")
            nc.scalar.dma_start(f_t[:], fmat[:])
            g_t = cpool.tile([96, 128], BF16, name="g_t")
            nc.scalar.dma_start(g_t[:], gmat[:])
            w_all = cpool.tile([128, NP * 128], BF16, name="w_all")
            nc.scalar.dma_start(w_all[:], wmid[:])

            # ---- software-pipelined emission: PE stream is
            # A0 A1 M0 M1 C0 C1, with regroup DMA data flowing behind the
            # PE. Regroup-2 writes + reads go on the otherwise idle GPSIMD
            # (SWDGE) queue; sync carries x/sW/gR/out; ACT and DVE split
            # the PSUM->SBUF copies via single-writer tiles.
            # HAM warm-up: the PE clock gate opens only after ~3.4us of
            # sustained activity; burn idle boot time on throwaway matmuls
            # so stage A starts at full clock. No reader consumes ps_w.
            ps_w = psum.tile([96, MMG * BC], mybir.dt.float32,
                             name="ps_w", tag="ps")
            for _ in range(28):
                nc.tensor.matmul(ps_w[:, :96], f_t[:], f_t[:, :96],
                                 start=True, stop=True)

            x_g = {}
            for c in range(CH):
                for g in range(KI // JG):
                    xt = xpool.tile([128, JG * BC], BF16, name="x_g",
                                    tag="x_g")
                    col0 = (c * KI + g * JG) * BC
                    nc.sync.dma_start(xt[:], xP[:, col0:col0 + JG * BC])
                    x_g[c, g] = xt

            s_t = {}

            def stage_a(c):
                s_lo = spool.tile([96, JH * BC], BF16, name="s_lo",
                                  tag="s_lo")
                s_hi = spool.tile([96, JH * BC], BF16, name="s_hi",
                                  tag="s_hi")
                for p4 in range(KI // MMG):
                    ps = psum.tile([96, MMG * BC], mybir.dt.float32,
                                   name="ps_a", tag="ps")
                    for k in range(MMG // 2):
                        j = MMG * p4 + 2 * k
                        rhs = x_g[c, j // JG][
                            :, (j % JG) * BC:(j % JG + 2) * BC]
                        nc.tensor.matmul(ps[:, 2 * k * BC:(2 * k + 2) * BC],
                                         f_t[:], rhs, start=True, stop=True)
                    half = KI // MMG // 2
                    if p4 < half:
                        nc.vector.tensor_copy(
                            s_lo[:, p4 * MMG * BC:(p4 + 1) * MMG * BC],
                            ps[:])
                    else:
                        c0 = (p4 - half) * MMG * BC
                        nc.scalar.copy(s_hi[:, c0:c0 + MMG * BC], ps[:])
                s_t[c] = (s_lo, s_hi)

            def emit_sw(c):
                eng = nc.sync
                for hh, sh in enumerate(s_t[c]):
                    for flp in range(4):
                        eng.dma_start(
                            sS_w[c, flp, :, hh * JH:(hh + 1) * JH],
                            sh[flp * 24:(flp + 1) * 24, :].rearrange(
                                "e (j b) -> e j b", j=JH))

            g_q = {}

            def emit_gr(c):
                eng = nc.sync
                for qd in range(NP // EQ):
                    gq = gpool.tile([128, EQ * BC], BF16, name="g_q",
                                    tag="g_q")
                    col0 = (c * NP + qd * EQ) * BC
                    eng.dma_start(gq[:], sS_g[:, col0:col0 + EQ * BC])
                    g_q[c, qd] = gq

            m_t = {}

            def stage_m(c):
                m_lo = mpool.tile([128, EH * BC], BF16, name="m_lo",
                                  tag="m_lo")
                m_hi = mpool.tile([128, EH * BC], BF16, name="m_hi",
                                  tag="m_hi")
                for p4 in range(NP // MMG):
                    ps = psum.tile([128, MMG * BC], mybir.dt.float32,
                                   name="ps_m", tag="ps")
                    for k in range(MMG):
                        e = MMG * p4 + k
                        gq = g_q[c, e // EQ]
                        # W_e is fl-block-diagonal: run the two 64-wide
                        # halves concurrently in disjoint PE quadrants
                        for fl in range(2):
                            r0 = fl * 64
                            rhs = gq[r0:r0 + 64,
                                     (e % EQ) * BC:(e % EQ + 1) * BC]
                            lhsT = w_all[r0:r0 + 64,
                                         e * 128 + r0:e * 128 + r0 + 64]
                            nc.tensor.matmul(
                                ps[r0:r0 + 64, k * BC:(k + 1) * BC],
                                lhsT, rhs, start=True, stop=True,
                                tile_position=(r0, r0))
                    half = NP // MMG // 2
                    if p4 < half:
                        nc.vector.tensor_copy(
                            m_lo[:, p4 * MMG * BC:(p4 + 1) * MMG * BC],
                            ps[:])
                    else:
                        c0 = (p4 - half) * MMG * BC
                        nc.scalar.copy(m_hi[:, c0:c0 + MMG * BC], ps[:])
                m_t[c] = (m_lo, m_hi)

            def emit_mw(c):
                for hh, mh in enumerate(m_t[c]):
                    for flq in range(4):
                        nc.gpsimd.dma_start(
                            sM_w[c, flq, :, hh * EH:(hh + 1) * EH],
                            mh[flq * 32:(flq + 1) * 32, :].rearrange(
                                "i (e b) -> i e b", e=EH))

            h_q = {}

            def emit_hr(c):
                for qd in range(KO // IQ):
                    hq = hpool.tile([96, IQ * BC], BF16, name="h_q",
                                    tag="h_q")
                    col0 = (c * KO + qd * IQ) * BC
                    nc.gpsimd.dma_start(hq[:],
                                        sM_h[:, col0:col0 + IQ * BC])
                    h_q[c, qd] = hq

            def stage_c(c):
                o_g = None
                for p4 in range(KO // MMG):
                    if (p4 * MMG) % JG == 0:
                        o_g = opool.tile([128, JG * BC], BF16, name="o_g",
                                         tag="o_g")
                    ps = psum.tile([128, MMG * BC], mybir.dt.float32,
                                   name="ps_c", tag="ps")
                    for k in range(MMG // 2):
                        i = MMG * p4 + 2 * k
                        rhs = h_q[c, i // IQ][
                            :, (i % IQ) * BC:(i % IQ + 2) * BC]
                        nc.tensor.matmul(ps[:, 2 * k * BC:(2 * k + 2) * BC],
                                         g_t[:], rhs, start=True, stop=True)
                    c0 = ((p4 * MMG) % JG) * BC
                    if ((p4 * MMG) // JG) % 2 == 0:
                        nc.vector.tensor_copy(o_g[:, c0:c0 + MMG * BC],
                                              ps[:])
                    else:
                        nc.scalar.copy(o_g[:, c0:c0 + MMG * BC], ps[:])
                    if ((p4 + 1) * MMG) % JG == 0:
                        col0 = (c * KO + (p4 + 1) * MMG - JG) * BC
                        nc.sync.dma_start(oP[:, col0:col0 + JG * BC],
                                          o_g[:])

            stage_a(0)
            emit_sw(0)
            stage_a(1)
            emit_gr(0)
            stage_m(0)
            emit_sw(1)
            emit_mw(0)
            emit_gr(1)
            stage_m(1)
            emit_hr(0)
            stage_c(0)
            emit_mw(1)
            emit_hr(1)
            stage_c(1)
    nc.finalize()
    return nc


def _get_nc():
    if "nc" not in _CACHE:
        _CACHE["nc"] = _build_nc()
    return _CACHE["nc"]


def _host_weights(W_real, W_imag):
    """F [128,96], G [96,128], Wmid [24,128,128] (float64)."""
    t = np.arange(B)[:, None].astype(np.float64)
    # F columns ordered (fl, p, e): f = 2e + fl; p=0 -> cos, p=1 -> -sin.
    F = np.zeros((128, 96))
    for fl in range(2):
        for p in range(2):
            for e in range(NP):
                f = 2 * e + fl
                col = fl * 48 + p * 24 + e
                w = 2 * np.pi * f * t[:, 0] / B
                F[:, col] = np.cos(w) if p == 0 else -np.sin(w)
    # G rows ordered (fl, q, e) with f = 2e + fl: q=0 -> scale*cos,
    # q=1 -> -scale*sin
    G = np.zeros((96, 128))
    scale = np.full(KT, 2.0 / B)
    scale[0] = 1.0 / B
    for fl in range(2):
        for q in range(2):
            for e in range(NP):
                f = 2 * e + fl
                w = 2 * np.pi * f * np.arange(B) / B
                G[fl * 48 + q * 24 + e] = (scale[f] * np.cos(w) if q == 0
                                           else -scale[f] * np.sin(w))
    # Wmid[e]: rows (fl, p, j), cols (fl, q, i); block-diag in fl
    Wr = W_real.astype(np.float64)
    Wi = W_imag.astype(np.float64)
    Wm = np.zeros((NP, 128, 128))
    for e in range(NP):
        for fl in range(2):
            f = 2 * e + fl
            r0, c0 = fl * 64, fl * 64
            # q=0: Re_out = Wr @ Re + Wi @ Im ; q=1: Im_out = Wr @ Im - Wi @ Re
            Wrf = Wr[:, :, f].T  # [j, i]
            Wif = Wi[:, :, f].T
            Wm[e, r0:r0 + 32, c0:c0 + 32] = Wrf            # p0 -> q0: Wr
            Wm[e, r0 + 32:r0 + 64, c0:c0 + 32] = Wif       # p1 -> q0: Wi
            Wm[e, r0:r0 + 32, c0 + 32:c0 + 64] = -Wif      # p0 -> q1: -Wi
            Wm[e, r0 + 32:r0 + 64, c0 + 32:c0 + 64] = Wrf  # p1 -> q1: Wr
    return F, G, Wm


def kernel(x, W_real, W_imag):
    global LAST_RESULTS
    import ml_dtypes
    from concourse.bass_utils import run_bass_kernel_spmd

    bf16 = np.dtype(ml_dtypes.bfloat16)
    x = np.asarray(x, dtype=np.float32)
    F, G, Wm = _host_weights(np.asarray(W_real), np.asarray(W_imag))
    Fb = np.ascontiguousarray(F).astype(bf16)
    Gb = np.ascontiguousarray(G).astype(bf16)
    wm_packed = np.ascontiguousarray(
        Wm.transpose(1, 0, 2).reshape(128, NP * 128)).astype(bf16)

    # pack x: [batch, (j t)] -> per core [t, (c, j, b)]
    xr = x.reshape(N_CORES, CH, BC, KI, B).transpose(0, 4, 1, 3, 2)
    xr = np.ascontiguousarray(xr.reshape(N_CORES, B, CH * KI * BC)).astype(
        bf16)

    in_maps = []
    for core in range(N_CORES):
        in_maps.append({"xP": xr[core], "fmat": Fb, "gmat": Gb,
                        "wmid": wm_packed})

    nc = _get_nc()
    res = run_bass_kernel_spmd(nc, in_maps, list(range(N_CORES)), trace=TRACE)
    LAST_RESULTS = res

    out = np.empty((BATCH, OUT_F), np.float32)
    for core in range(N_CORES):
        oPc = np.asarray(res.results[core]["oP"]).astype(np.float32)
        # [t, (c, i, b)] -> [(c b), (i t)]
        oc = oPc.reshape(B, CH, KO, BC).transpose(1, 3, 2, 0)
        out[core * BQ:(core + 1) * BQ, :] = oc.reshape(BQ, OUT_F)
    return out
